# revision 9
# baseline (speedup 1.0000x reference)
"""MoE routing kernel for Trainium2 (8 NeuronCores, expert-parallel).

Problem: y[n] = x[n] @ W[index[n]].T + b[index[n]]
  x [16384, 1024] f32, index [16384] i32, W [8, 512, 1024] f32, b [8, 512] f32

Strategy (expert-parallel, dispatch on index during sharding):
  Core e owns expert e. The host groups rows by expert (the all-to-all
  dispatch), packs each core's rows into PE-friendly transposed tiles, and
  each core runs a dense [R,1024] @ [1024,512] matmul with its expert's
  weights. Results are scattered back to original row order on the host.

Schedule notes (why the DMA issue order below looks odd):
  The runtime releases the 5 engine queues staggered and in a stable
  order: PE at ~0us, DVE ~4.1us, Pool(gpsimd) ~6.9us, Act(scalar) and
  SP(sync) ~8.5us; each engine then runs ~2.5us of fixed queue-start
  work before reaching the body. Only gpsimd/scalar/sync can trigger
  DMAs, so the first bytes can move at ~10.3us (gpsimd) / ~11.5us
  (scalar, sync). The init all-engine barrier is skipped so the early
  engines do not wait for the late ones, and the first DMAs are issued
  from gpsimd in consumption-deadline order, interleaved across the
  three ~128GB/s dynamic DMA queues. The head x block is packed on the
  host as one contiguous 128KB chunk per k-tile so each critical DMA is
  a single clean descriptor set.

Device layout per core (one NEFF, SPMD on cores 0-7):
  xh  [KT, 128, H*128]   (k-tile, k%128, head-row-tile*128 + r) — head
                         lhsT chunks, contiguous per k-tile
  xr  [RT-H, 128, 8*128] (row-tile, k%128, k-tile*128 + r) — rest lhsT
  wT  [8, 128, 512]      (k-tile, k, o) — rhs blocks (moving)
  y   [RT, 128, 512]     (row-tile, r, o)
  Head: k-major over the first H row-tiles (matches W/xh arrival rate).
  Rest: per row-tile, accumulate 8 matmuls over k-tiles into one PSUM
  bank, copy PSUM->SBUF on DVE, DMA out.
"""

from contextlib import ExitStack

import numpy as np

import concourse.bass as bass
import concourse.mybir as mybir
import concourse.tile as tile
from concourse import bacc
from concourse.bass_utils import run_bass_kernel_spmd

N_CORES = 8
D_IN = 1024
D_OUT = 512
KT = D_IN // 128  # 8 k-tiles
H = 8  # head row-tiles processed k-major

# matmul input dtypes (lhsT = x blocks, rhs = W blocks). float16 runs the
# PE at 1 column/cycle with fast weight load (fp32 is 4x slower, fp32r has
# no fast weight load) and halves the input DMA. Accuracy vs the fp32
# reference is ~5e-4 relative (10-bit mantissa; values here are well within
# fp16 range: |x| < ~6, |W| < ~0.06, accumulation in fp32 PSUM).
X_DT = mybir.dt.float16
W_DT = mybir.dt.float16

# Output DMA dtype. float16 halves the store traffic; the host upcasts
# back to float32. Adds at most 2^-11 relative rounding.
Y_DT = mybir.dt.float16

# PE-warmup dummy matmuls. The HAM clock gate keeps the PE at 1.2 GHz
# until it has been busy ~3.4us (8 x 512-col fp16 matmuls at 1.2 GHz),
# and re-throttles after ~3.4us idle. The warmup runs right after DVE's
# memset (~6.6us), well before the first data arrives (~12us), so the
# real stream runs at 2.4 GHz from its first matmul.
WARMUP_MMS = 3

# Skip the construction-time all-engine barrier. Its only job is to order
# the const-pool memsets (which this kernel never reads) before the body;
# skipping it lets each engine enter the body as soon as the runtime
# releases it instead of waiting for the last engine (~10us after the
# first). All body dependencies are still managed by Tile's semaphores
# (initialized by the NEFF loader, not by engine code).
SKIP_INIT_BARRIER = True


class _NoInitBarrierBacc(bacc.Bacc):
    """Bacc whose construction-time all-engine barrier is skipped."""

    def all_engine_barrier(self, *, sem_only: bool = False):
        if not getattr(self, "_init_barrier_skipped", False):
            self._init_barrier_skipped = True
            return None
        return super().all_engine_barrier(sem_only=sem_only)


def build_nc(rt: int, x_dt=None, w_dt=None):
    """Build + compile the per-core Bass program for `rt` row-tiles."""
    x_dt = x_dt or X_DT
    w_dt = w_dt or W_DT
    h = min(H, rt)
    nc = (_NoInitBarrierBacc if SKIP_INIT_BARRIER else bacc.Bacc)(
        "TRN2",
        target_bir_lowering=False,
        debug=False,
        enable_asserts=False,
        num_devices=N_CORES,
    )
    f32 = mybir.dt.float32
    xh = nc.dram_tensor("xh", [KT, 128, h * 128], x_dt, kind="ExternalInput").ap()
    if rt > h:
        xr = nc.dram_tensor(
            "xr", [rt - h, 128, KT * 128], x_dt, kind="ExternalInput"
        ).ap()
    wT = nc.dram_tensor("wT", [KT, 128, D_OUT], w_dt, kind="ExternalInput").ap()
    y = nc.dram_tensor("y", [rt, 128, D_OUT], Y_DT, kind="ExternalOutput").ap()

    with tile.TileContext(nc) as tc, ExitStack() as ctx:
        w_pool = ctx.enter_context(tc.tile_pool(name="w", bufs=1))
        xh_pool = ctx.enter_context(tc.tile_pool(name="xh", bufs=1))
        x_pool = ctx.enter_context(tc.tile_pool(name="x", bufs=max(rt - h, 1)))
        o_pool = ctx.enter_context(tc.tile_pool(name="o", bufs=8))
        p_pool = ctx.enter_context(tc.tile_pool(name="p", bufs=8, space="PSUM"))
        warm_pool = ctx.enter_context(tc.tile_pool(name="warm", bufs=1))

        w_sb = w_pool.tile([128, KT * D_OUT], w_dt, tag="w", name="w_sb")
        w_tiles = [w_sb[:, kt * D_OUT : (kt + 1) * D_OUT] for kt in range(KT)]
        xh_sb = xh_pool.tile([128, KT * h * 128], x_dt, tag="xh", name="xh")

        # PE warmup (see WARMUP_MMS). Starts as soon as DVE's memset
        # lands (~13.6us); just enough dummies to bridge until the first
        # data arrives (~14.6us) — extra ones would displace the real
        # matmuls, and the HAM busy accumulation pauses (not resets) on
        # short idles. All PSUM tiles share one pool/tag so the 8 banks
        # cycle: warm, 8 head psums (the last reuses the warm bank), then
        # the row tiles.
        warm_sb = warm_pool.tile([128, D_OUT], x_dt, tag="warm", name="warm_sb")
        nc.vector.memset(warm_sb[:], 0.0)
        warm_ps = p_pool.tile([128, D_OUT], f32, tag="ps", name="warm_ps")
        for i in range(WARMUP_MMS):
            nc.tensor.matmul(
                warm_ps[:], warm_sb[:, :128], warm_sb[:], start=True, stop=True
            )

        # Critical-path DMA issues, ordered by engine wake time (gpsimd
        # first) and by consumption deadline, interleaved across the three
        # dynamic DMA queues (one per engine, ~128GB/s each). The head
        # consumes w[kt] + xh chunk kt every H*216ns once the stream
        # starts (~12.4us); each 128KB transfer occupies its queue ~1us.
        def xh_chunk(kt):
            return xh_sb[:, kt * h * 128 : (kt + 1) * h * 128]

        def xh_half(kt, hh):
            q = h // 2
            lo = kt * h * 128 + hh * q * 128
            return (
                xh_sb[:, lo : lo + q * 128],
                xh[kt][:, hh * q * 128 : (hh + 1) * q * 128],
            )

        def w_range(k0, k1):
            dst = w_sb[:, k0 * D_OUT : k1 * D_OUT]
            src = wT[k0:k1]
            if k1 - k0 > 1:
                dst = dst.rearrange("p (c f) -> p c f", c=k1 - k0)
                src = src.rearrange("c p f -> p c f")
            else:
                src = src.rearrange("c p f -> (c p) f")
            return dst, src

        g, a, s = nc.gpsimd, nc.scalar, nc.sync
        # Tile tracks HWDGE DMA completion on 8 round-robin semaphore
        # lanes shared by scalar+sync; a 9th in-flight HWDGE DMA blocks
        # its issuing engine until the lane's previous transfer retires.
        # So the initial burst is exactly 8 HWDGE transfers (sized up to
        # 512KB), ordered by consumption deadline across the two ~128GB/s
        # queues; gpsimd (separate SWDGE lanes + queue) carries xh4/xh6
        # and the first two rest tiles.
        s.dma_start(*xh_half(0, 0))   # L0  kt0 j0-3   ~14.4us
        s.dma_start(*xh_half(0, 1))   # L1  kt0 j4-7   ~15.4
        a.dma_start(*w_range(0, 1))   # L2  w0         ~14.7
        a.dma_start(xh_chunk(1), xh[1])  # L3 kt1      ~16.8
        s.dma_start(*w_range(1, 3))   # L4  w1,w2      ~17.5
        a.dma_start(xh_chunk(2), xh[2])  # L5 kt2      ~18.8
        s.dma_start(*w_range(3, 5))   # L6  w3,w4      ~19.5
        a.dma_start(xh_chunk(3), xh[3])  # L7 kt3      ~20.9
        g.dma_start(xh_chunk(4), xh[4])  # SW          ~16.5
        g.dma_start(xh_chunk(6), xh[6])  # SW          ~18.5
        s.dma_start(xh_chunk(5), xh[5])  # L0          ~22.9
        a.dma_start(*w_range(5, 7))   # L1  w5,w6      ~22.0
        s.dma_start(*w_range(7, 8))   # L2  w7         ~23.9
        a.dma_start(xh_chunk(7), xh[7])  # L3          ~25.1

        # Rest-tile loads: first two on the idle SWDGE queue, the rest
        # alternating the two HWDGE queues.
        x_tiles = {}
        for i, r in enumerate(range(h, rt)):
            x_t = x_pool.tile([128, KT * 128], x_dt, tag="x", name=f"x{r}")
            eng = g if i < 2 else (s if i % 2 == 0 else a)
            eng.dma_start(x_t[:], xr[r - h])
            x_tiles[r] = x_t

        def store_out(r, psum):
            o_t = o_pool.tile([128, D_OUT], Y_DT, tag="o", name=f"o{r}")
            if r == rt - 1:
                # Copy + store in halves on separate queues: the second
                # half's chain (copy -> issue -> transfer -> receipt) is
                # what trails the last matmul; halving and splitting it
                # shortens the kernel tail.
                half = D_OUT // 2
                for hh, eng in ((0, a), (1, s)):
                    sl = slice(hh * half, (hh + 1) * half)
                    nc.vector.tensor_copy(o_t[:, sl], psum[:, sl])
                    eng.dma_start(y[r][:, sl], o_t[:, sl])
            else:
                # Mid-stream stores ride the otherwise-idle gpsimd queue so
                # the x loads on scalar/sync are never stuck behind them;
                # the trailing two tiles go on the (by then idle) HWDGE
                # queues whose receipts don't lag like SWDGE's.
                nc.vector.tensor_copy(o_t[:], psum[:])
                eng = g if r < rt - 3 else (a if r == rt - 3 else s)
                eng.dma_start(y[r], o_t[:])

        # Head: k-major so one W k-tile + one xh chunk feed H matmuls.
        head_psums = [
            p_pool.tile([128, D_OUT], f32, tag="ps", name=f"ps{j}") for j in range(h)
        ]
        for kt in range(KT):
            for j in range(h):
                off = kt * h * 128 + j * 128
                nc.tensor.matmul(
                    head_psums[j][:],
                    xh_sb[:, off : off + 128],
                    w_tiles[kt][:],
                    start=(kt == 0),
                    stop=(kt == KT - 1),
                )
        for j in range(h):
            store_out(j, head_psums[j])

        for r in range(h, rt):
            x_t = x_tiles[r]
            psum = p_pool.tile([128, D_OUT], f32, tag="ps", name=f"ps{r}")
            for kt in range(KT):
                nc.tensor.matmul(
                    psum[:],
                    x_t[:, bass.ts(kt, 128)],
                    w_tiles[kt][:],
                    start=(kt == 0),
                    stop=(kt == KT - 1),
                )
            store_out(r, psum)

    nc.compile()
    return nc


def make_in_maps(x, index, W, x_dt=None, w_dt=None):
    """Group rows by expert, pack per-core transposed tiles.

    Returns (in_maps, rows_per_expert, rt) where rows_per_expert[e] is the
    original row indices handled by core e.
    """
    import concourse.mybir as _mybir

    x_np = _mybir.dt.np(x_dt or X_DT)
    w_np = _mybir.dt.np(w_dt or W_DT)
    x = np.ascontiguousarray(x, dtype=np.float32)
    W = np.ascontiguousarray(W, dtype=np.float32)
    rows_per_expert = [np.nonzero(index == e)[0] for e in range(N_CORES)]
    max_rows = max(len(r) for r in rows_per_expert)
    rt = max((max_rows + 127) // 128, 1)
    r_pad = rt * 128
    h = min(H, rt)

    in_maps = []
    for e in range(N_CORES):
        rows = rows_per_expert[e]
        xp = np.zeros((r_pad, D_IN), np.float32)
        xp[: len(rows)] = x[rows]
        # [R, D_IN] -> [RT, 128r, KT, 128k] -> [RT, 128k, KT, 128r]
        # so a partition line (fixed k) is KT*128 elements contiguous.
        xT = (
            xp.reshape(rt, 128, KT, 128).transpose(0, 3, 2, 1).reshape(rt, 128, -1)
        )
        # Head: one contiguous [128, h*128] chunk per k-tile.
        xh = np.ascontiguousarray(
            xT[0:h].reshape(h, 128, KT, 128).transpose(2, 1, 0, 3).reshape(KT, 128, -1),
            dtype=x_np,
        )
        wT = np.ascontiguousarray(W[e].T.reshape(KT, 128, D_OUT), dtype=w_np)
        m = {"xh": xh, "wT": wT}
        if rt > h:
            m["xr"] = np.ascontiguousarray(xT[h:], dtype=x_np)
        in_maps.append(m)
    return in_maps, rows_per_expert, rt


def assemble_output(results, rows_per_expert, n_rows, index=None, b=None):
    y = np.zeros((n_rows, D_OUT), np.float32)
    for e, rows in enumerate(rows_per_expert):
        yc = results[e]["y"].reshape(-1, D_OUT)
        y[rows] = yc[: len(rows)].astype(np.float32)
    if b is not None and np.any(b):
        y += np.asarray(b, np.float32)[np.asarray(index)]
    return y


def kernel(x, index, W, b):
    x = np.asarray(x)
    index = np.asarray(index, np.int32)
    W = np.asarray(W)
    b = np.asarray(b)
    in_maps, rows_per_expert, rt = make_in_maps(x, index, W)
    nc = build_nc(rt)
    res = run_bass_kernel_spmd(nc, in_maps, core_ids=list(range(N_CORES)))
    return assemble_output(res.results, rows_per_expert, x.shape[0], index, b)
